# revision 36
# baseline (speedup 1.0000x reference)
"""CoPE-style kernel for Trainium2 (8 NeuronCores, SPMD row-sharded).

Computation (matches the reference):
    pos_vecs = pos_emb / max(||pos_emb||_row, eps)          # [16, 4096]
    logits   = (q @ pos_vecs.T) / sqrt(4096)                # [B*T, 16]
    gates    = softmax(logits, axis=-1)
    out      = gates @ pos_vecs                             # [B*T, 4096]

Device strategy (per core, rows sharded 8 ways -> 2048 rows/core):
  - q is cast to bf16 on the host: halves the device's DMA-in traffic and
    enables hardware DMA-transpose (2-byte dtypes only). The rounding
    perturbs logits by ~3e-5 absolute, far inside fp32 tolerance.
  - q arrives in SBUF already transposed via the DMA xbar (one [512,128] ->
    [128,512] transpose-load per D-chunk), so the D-contraction needs zero
    Tensor-engine transposes and no PSUM round-trip for q.
  - logits^T [16, 512] accumulates on PE over 32 K-chunks per 512-row
    super-tile; exp runs on the Scalar engine with the 1/64 softmax scale
    folded into the activation.
  - e^T is the stationary matmul operand for BOTH the softmax denominator
    (rhs = ones) and the output projection (rhs = pos_vecs, fp32r: 1 PE
    cycle/row vs 4 for fp32, ~tf32 rounding -> ~1e-4 output error).
  - The 1/sum normalization is folded into the PSUM->SBUF evacuation copies
    (per-partition scalar multiply, alternating Vector/Scalar engines), so
    softmax costs no extra passes.
  - pos_emb normalization runs on-device: row sumsq via ACT Square+accum,
    sqrt + reciprocal + two Newton rsqrt refinements; the transposed
    codebook is built with PE matmuls against diag(1/norm).
  - q loads issue from the SP HWDGE ring, out stores from the Scalar ring,
    so loads and stores don't FIFO-serialize behind each other.
"""

import contextlib

import numpy as np
import ml_dtypes

import concourse.bacc as bacc
import concourse.mybir as mybir
import concourse.tile as tile
from concourse.bass_utils import run_bass_kernel_spmd
from concourse.alu_op_type import AluOpType

B, T, D = 4, 4096, 4096
N_POS = 16
N_CORES = 8
ROWS = B * T
ROWS_PER_CORE = ROWS // N_CORES          # 2048
SUP = 512                                # rows per super-tile
SUP_TILES = ROWS_PER_CORE // SUP         # 4
ROW_TILES = ROWS_PER_CORE // 128         # (kept for harness use)
D_CHUNKS = D // 128                      # 32
OUT_CHUNKS = D // 512                    # 8
SOFTMAX_SCALE = 1.0 / float(np.sqrt(D))  # 1/64 exactly

F32 = mybir.dt.float32
F32R = mybir.dt.float32r
BF16 = mybir.dt.bfloat16
AF = mybir.ActivationFunctionType

_CACHE = {}


def _build_kernel(tc, q_ap, pe_ap, i128_ap, i16_ap, ones_ap, out_ap, loop_reps=None):
    nc = tc.nc

    with (
        tc.tile_pool(name="const", bufs=1) as const_pool,
        tc.tile_pool(name="qt", bufs=8) as qt_pool,
        tc.tile_pool(name="et", bufs=4) as et_pool,
        tc.tile_pool(name="rec", bufs=4) as rec_pool,
        tc.tile_pool(name="outs", bufs=4) as outs_pool,
        # one shared 7-bank pool ("ps" tag) for the logits accumulators and
        # the mm2 outputs: the four lt banks are only held during the load
        # phase, so the tail's mm2/evacuation pipeline deepens from 3 to 7
        # banks as exp frees them
        tc.tile_pool(name="ps", bufs=8, space="PSUM") as ps_pool,
    ):
        # ---- constants ----
        i16 = const_pool.tile([N_POS, N_POS], F32)
        nc.sync.dma_start(i16[:], i16_ap[:])
        # [16, 2] (not [16, 1]): fp32r matmuls require an even moving-dim
        ones = const_pool.tile([N_POS, 2], F32R)
        nc.sync.dma_start(ones[:], ones_ap[:])
        pe_s = const_pool.tile([N_POS, D], F32)
        nc.sync.dma_start(pe_s[:], pe_ap[:])

        # ---- normalize codebook on device ----
        sq = const_pool.tile([N_POS, D], F32)
        ss = const_pool.tile([N_POS, 1], F32)
        nc.scalar.activation(sq[:], pe_s[:], AF.Square, accum_out=ss[:])
        norm0 = const_pool.tile([N_POS, 1], F32)
        nc.scalar.activation(norm0[:], ss[:], AF.Sqrt)
        r = const_pool.tile([N_POS, 1], F32)
        nc.vector.reciprocal(r[:], norm0[:])
        # two Newton steps: r <- r * (1.5 - 0.5*ss*r^2); ACT sqrt has a loose
        # ULP budget, this brings rsqrt to fp32 roundoff regardless
        for it in range(2):
            t1 = const_pool.tile([N_POS, 1], F32, name=f"nt1_{it}")
            nc.vector.tensor_mul(t1[:], r[:], r[:])
            t2 = const_pool.tile([N_POS, 1], F32, name=f"nt2_{it}")
            nc.vector.tensor_mul(t2[:], t1[:], ss[:])
            t3 = const_pool.tile([N_POS, 1], F32, name=f"nt3_{it}")
            nc.vector.tensor_scalar(t3[:], t2[:], -0.5, 1.5, AluOpType.mult, AluOpType.add)
            rn = const_pool.tile([N_POS, 1], F32, name=f"nr_{it}")
            nc.vector.tensor_mul(rn[:], t3[:], r[:])
            r = rn
        # d16 = diag(1/norm)
        d16 = const_pool.tile([N_POS, N_POS], F32)
        nc.vector.tensor_scalar_mul(d16[:], i16[:], r[:])
        # pv = normalized codebook [16, D], fp32r (rhs of the output matmul)
        pv = const_pool.tile([N_POS, D], F32R)
        nc.vector.tensor_scalar_mul(pv[:], pe_s[:], r[:])
        # pvt = normalized codebook transposed+folded, bf16 [128, 32*16]:
        # pvt[p, 16c+n] = pos_vecs[n, 128c+p]; built via PE transpose-matmuls
        # against diag(1/norm) (fuses the transpose and the normalization)
        pvt_psum = ps_pool.tile([128, 512], F32, tag="ps")
        for c in range(D_CHUNKS):
            nc.tensor.matmul(
                pvt_psum[:, c * 16:(c + 1) * 16],
                lhsT=pe_s[:, c * 128:(c + 1) * 128],
                rhs=d16[:],
                start=True, stop=True,
            )
        pvt = const_pool.tile([128, D_CHUNKS * N_POS], BF16)
        nc.vector.tensor_copy(pvt[:], pvt_psum[:])

        # ---- main loop over 512-row super-tiles ----
        # loop_reps is a timing-harness hook: it repeats the whole pass inside
        # a device-side For_i so per-pass HW time can be isolated from host
        # dispatch overhead. The graded path uses loop_reps=None.
        # The xbar-mode HW constraint serializes transpose-DMAs against
        # copy-DMAs, so the pass is phased: a load phase (32 transpose-loads,
        # mm1 hidden behind them) followed by a store phase (mm2/evacuation
        # hidden behind the 16 output stores).
        rep_ctx = tc.For_i(0, loop_reps, 1) if loop_reps else contextlib.nullcontext()
        with rep_ctx:
            # logits^T accumulators [16 pos, 512 rows], one PSUM bank per
            # super-tile, all four accumulated in parallel so each q chunk is
            # consumed (and its SBUF slot freed) immediately after arrival
            lts = [
                ps_pool.tile([N_POS, SUP], F32, tag="ps", name=f"lt{s}")
                for s in range(SUP_TILES)
            ]
            for c in range(D_CHUNKS):
                # whole-column transpose-load via the DMA xbar:
                # qt[d, r] = q[r, 128c + d] for all 2048 rows
                qt = qt_pool.tile([128, ROWS_PER_CORE], BF16)
                # single HWDGE ring for transpose-loads: two rings would be
                # ~25% faster but concurrent DMA-transposes corrupt each other
                # through the shared xbar unit (observed as nondeterministic
                # ~1e-4 output errors)
                nc.sync.dma_start(
                    qt[:], q_ap[:, c * 128:(c + 1) * 128], transpose=True,
                )
                for s in range(SUP_TILES):
                    nc.tensor.matmul(
                        lts[s][:],
                        lhsT=pvt[:, c * 16:(c + 1) * 16],
                        rhs=qt[:, s * SUP:(s + 1) * SUP],
                        start=(c == 0), stop=(c == D_CHUNKS - 1),
                    )
            # softmax stats for all four super-tiles up front, so the DVE
            # reciprocals don't queue behind evacuation copies (DVE is FIFO)
            ets, recs = [], []
            for s in range(SUP_TILES):
                # e^T = exp(logits^T / 64); no max-subtraction needed:
                # |logits/64| <= ~0.1 for unit-norm codebook rows (softmax is
                # shift-invariant, exp can't overflow here)
                et = et_pool.tile([N_POS, SUP], F32R, tag="et", name=f"et{s}")
                nc.scalar.activation(et[:], lts[s][:], AF.Exp, scale=SOFTMAX_SCALE)
                # softmax denominators, one 128-row block per matmul; the sums
                # bank comes from the shared pool (a slot just freed by exp)
                sums = ps_pool.tile([128, 8], F32, tag="ps")
                for b in range(4):
                    nc.tensor.matmul(
                        sums[:, 2 * b:2 * b + 2],
                        lhsT=et[:, b * 128:(b + 1) * 128],
                        rhs=ones[:],
                        start=True, stop=True,
                    )
                rec = rec_pool.tile([128, 8], F32, tag="rec", name=f"rec{s}")
                nc.vector.reciprocal(rec[:], sums[:])
                ets.append(et)
                recs.append(rec)

            for s in range(SUP_TILES):
                r0 = s * SUP
                et, rec = ets[s], recs[s]
                for b in range(4):
                    outs = outs_pool.tile([128, D], F32)
                    for k in range(OUT_CHUNKS):
                        op = ps_pool.tile([128, 512], F32, tag="ps")
                        nc.tensor.matmul(
                            op[:],
                            lhsT=et[:, b * 128:(b + 1) * 128],
                            rhs=pv[:, k * 512:(k + 1) * 512],
                            start=True, stop=True,
                        )
                        # evacuate PSUM with the softmax normalization folded in
                        dst = outs[:, k * 512:(k + 1) * 512]
                        if k % 2 == 0:
                            nc.vector.tensor_scalar_mul(dst, op[:], rec[:, 2 * b:2 * b + 1])
                        else:
                            nc.scalar.activation(dst, op[:], AF.Copy, scale=rec[:, 2 * b:2 * b + 1])
                    # stores also alternate the two HWDGE rings; they can't
                    # overlap the transpose-loads (xbar-mode serialization)
                    # but the load and store phases are disjoint anyway
                    seng = nc.sync if (s * 4 + b) % 2 == 0 else nc.scalar
                    seng.dma_start(
                        out_ap[r0 + b * 128:r0 + (b + 1) * 128, :], outs[:]
                    )


def _get_nc():
    if "nc" in _CACHE:
        return _CACHE["nc"]
    nc = bacc.Bacc("TRN2", debug=False, num_devices=N_CORES)
    q_d = nc.dram_tensor("q", [ROWS_PER_CORE, D], BF16, kind="ExternalInput")
    pe_d = nc.dram_tensor("pos_emb", [N_POS, D], F32, kind="ExternalInput")
    i128_d = nc.dram_tensor("ident128", [128, 128], BF16, kind="ExternalInput")
    i16_d = nc.dram_tensor("ident16", [N_POS, N_POS], F32, kind="ExternalInput")
    ones_d = nc.dram_tensor("ones16", [N_POS, 2], F32R, kind="ExternalInput")
    out_d = nc.dram_tensor("out", [ROWS_PER_CORE, D], F32, kind="ExternalOutput")
    with tile.TileContext(nc) as tc:
        _build_kernel(
            tc, q_d.ap(), pe_d.ap(), i128_d.ap(), i16_d.ap(), ones_d.ap(), out_d.ap()
        )
    nc.compile()
    _CACHE["nc"] = nc
    return nc


def _make_in_maps(q, pos_emb):
    # host-side bf16 ingest of q (see module docstring)
    qf = np.asarray(q, dtype=np.float32).reshape(ROWS, D).astype(ml_dtypes.bfloat16)
    pe = np.ascontiguousarray(np.asarray(pos_emb, dtype=np.float32))
    i128 = np.eye(128, dtype=ml_dtypes.bfloat16)
    i16 = np.eye(N_POS, dtype=np.float32)
    ones = np.ones((N_POS, 2), dtype=np.float32)
    return [
        {
            "q": qf[c * ROWS_PER_CORE:(c + 1) * ROWS_PER_CORE],
            "pos_emb": pe,
            "ident128": i128,
            "ident16": i16,
            "ones16": ones,
        }
        for c in range(N_CORES)
    ]


def kernel(q, x, pos_emb):
    nc = _get_nc()
    in_maps = _make_in_maps(q, pos_emb)
    res = run_bass_kernel_spmd(nc, in_maps, list(range(N_CORES)))
    out = np.concatenate([res.results[c]["out"] for c in range(N_CORES)], axis=0)
    return out.reshape(B, T, D).astype(np.float32, copy=False)
